# revision 48
# baseline (speedup 1.0000x reference)
"""Angular-select masked-FFT kernel for Trainium2 (8 NeuronCores, data-parallel over batch).

Math: per (b, g): diff[w] = sum_h ||re0|-|im1|| + ||re1|-|im0||; select 64 smallest w;
out = fft_w(ifft_h(x masked to sel columns)) + 0.5, emitted as interleaved re/im.

Two dense global phases over the 8 (b,g) units per core; stages are emitted
oldest-unit-first so the in-order engine streams never head-of-line block, the
SP DGE carries only input loads + output stores, and the ACT DGE carries the
small scratch round trips.

Phase 1 (input streaming + selection), unit-pipelined 7 deep:
  u+0  input load (sync DGE): u16-quantized (max,min) |x| pairs, 8KB/partition
       descriptors. Quantization (round(|x|*5957)) keeps every diff integer-
       exact (< 2^24, verified 0 selection flips, min boundary gap 21 ulp) at
       half the f32 traffic, and the host-side (max,min) ordering makes
       ||a|-|b|| a single u16 subtract (2x DVE rate, no abs pass).
  u+1  whole diff chain on DVE (no cross-engine hops): 2 subs + widening tree
       adds, then gpsimd partition_all_reduce -> broadcast diff row (f32-exact)
  u+2  dscr write + dcol read (ACT DGE round trip)
  u+3  rank[w] = #{d[w'] < d[w]} via is_lt+accum (DVE); mask = rank < 64
  u+4  vscr write + v16 read (ACT DGE)
  u+5  sparse_gather compacts selected w ids (gpsimd); idx lists (DVE)
  u+6  idx replication to 128 partitions (ACT DGE)

Phase 2 (gathers + FFTs as matmuls), unit-pipelined 3 deep:
  k+0  gathers (gpsimd DGE): gx 128 rows x 2KB transposed (re|im per row ->
       h-chunk lhsT layout), gc 65 rows x 4KB (C1|C2 DFT rows + bias row
       folding the +0.5)
  k+1  stage B ifft_h: both channels packed in one 128-row PSUM pair, 16
       matmuls, stationary operand reused across consecutive matmuls (half the
       LDWEIGHTS); contiguous evict to bf16 Y^T tiles [65,128,4] (h'=4p+j),
       bias rows persistent in parity-alternated tiles
  k+2  stage C fft_w: contraction 65 (64 sel + bias row), interleaved re/im
       output; evict to bf16 staging laid out so the store writes
       8KB/partition descriptors
  k+3  output store (sync DGE)

Output is bf16, upcast to f32 on host (error budget 2e-2 >> bf16 rounding).
"""

import os
import sys
from contextlib import ExitStack

import numpy as np

sys.path.insert(0, "/opt/trn_rl_repo")

B, C, H, W = 32, 4, 512, 512
T = 64
G = 2
NCORES = 8
BPC = B // NCORES  # samples per core

_cache = {}


def _build_consts():
    import ml_dtypes

    bf = ml_dtypes.bfloat16
    h = np.arange(H, dtype=np.float64)
    th = 2.0 * np.pi * np.outer(h, h) / H
    cos_i = (np.cos(th) / H).astype(np.float32)
    sin_i = (np.sin(th) / H).astype(np.float32)
    w = np.arange(W, dtype=np.float64)
    tw = 2.0 * np.pi * np.outer(w, w) / W
    fr = np.cos(tw).astype(np.float32)
    fi = (-np.sin(tw)).astype(np.float32)
    # row w = [C1_w (1024 interleaved re/im), C2_w (1024)]; row W = bias (+0.5 re)
    cb = np.zeros((W + 1, 2, 2 * W), np.float32)
    cb[:W, 0, 0::2] = fr
    cb[:W, 0, 1::2] = fi
    cb[:W, 1, 0::2] = -fi
    cb[:W, 1, 1::2] = fr
    cb[W, 0, 0::2] = 0.5
    return (
        cos_i.astype(bf),
        sin_i.astype(bf),
        (-sin_i).astype(bf),
        np.ascontiguousarray(cb.reshape(W + 1, 4 * W)).astype(bf),
    )


def _build_kernel():
    import concourse.bass as bass
    import concourse.tile as tile
    from concourse import bacc, bass_isa, mybir

    f32 = mybir.dt.float32
    bf16 = mybir.dt.bfloat16
    i16 = mybir.dt.int16
    i32 = mybir.dt.int32
    u32 = mybir.dt.uint32
    u16 = mybir.dt.uint16
    u8 = mybir.dt.uint8
    Alu = mybir.AluOpType
    Act = mybir.ActivationFunctionType
    MASK = 0x7FFFFFFF

    nc = bacc.Bacc("TRN2", target_bir_lowering=False, debug=False, num_devices=NCORES)

    # quantized |x| pre-paired as (max, min): pair0 = (|re0|,|im1|), pair1 =
    # (|re1|,|im0|); values = round(|x| * 5957) in u16 so the per-element
    # ||a|-|b|| is ONE subtract and all integer sums stay exact (< 2^24)
    axp = nc.dram_tensor("axp", [BPC, G, 2, 2, H, W], u16, kind="ExternalInput").ap()
    # transposed bf16: row (c*W + w) = [x_re[b,c,:,w] (H), x_im[b,c,:,w] (H)]
    x16 = nc.dram_tensor("x16", [BPC, C * W, 2 * H], bf16, kind="ExternalInput").ap()
    cos_d = nc.dram_tensor("cos_i", [H, H], bf16, kind="ExternalInput").ap()
    sin_d = nc.dram_tensor("sin_i", [H, H], bf16, kind="ExternalInput").ap()
    nsin_d = nc.dram_tensor("nsin_i", [H, H], bf16, kind="ExternalInput").ap()
    cb_d = nc.dram_tensor("cboth", [W + 1, 4 * W], bf16, kind="ExternalInput").ap()
    out_d = nc.dram_tensor("out", [BPC, C, H, W, 2], bf16, kind="ExternalOutput").ap()
    dscr = nc.dram_tensor("dscr", [BPC, G, 512], f32).ap()
    vscr = nc.dram_tensor("vscr", [BPC, G, 512], f32).ap()

    with tile.TileContext(nc) as tc, ExitStack() as ctx:
        const_pool = ctx.enter_context(tc.tile_pool(name="consts", bufs=1))
        xpool = ctx.enter_context(tc.tile_pool(name="x", bufs=3))
        vpool = ctx.enter_context(tc.tile_pool(name="vsmall", bufs=4))
        spool = ctx.enter_context(tc.tile_pool(name="small", bufs=4))
        ipool = ctx.enter_context(tc.tile_pool(name="idx", bufs=8))
        gxpool = ctx.enter_context(tc.tile_pool(name="gatherx", bufs=8))
        gcpool = ctx.enter_context(tc.tile_pool(name="gatherc", bufs=8))
        psum_y = ctx.enter_context(tc.tile_pool(name="psy", bufs=1, space="PSUM"))
        psum_o = ctx.enter_context(tc.tile_pool(name="pso", bufs=3, space="PSUM"))
        opool = ctx.enter_context(tc.tile_pool(name="ostage", bufs=3))

        # --- constants in SBUF ---
        sb_cos = const_pool.tile([128, 4, H], bf16)
        sb_sin = const_pool.tile([128, 4, H], bf16)
        sb_nsin = const_pool.tile([128, 4, H], bf16)
        nc.sync.dma_start(sb_cos[:], cos_d.rearrange("(a p) w -> p a w", p=128))
        nc.sync.dma_start(sb_sin[:], sin_d.rearrange("(a p) w -> p a w", p=128))
        nc.sync.dma_start(sb_nsin[:], nsin_d.rearrange("(a p) w -> p a w", p=128))
        iota_i = const_pool.tile([128, 4], i32)
        iota_f = const_pool.tile([128, 4], f32)
        nc.gpsimd.iota(iota_i[:], pattern=[[1, 4]], base=0, channel_multiplier=4)
        nc.vector.tensor_copy(iota_f[:], iota_i[:])
        # persistent Y^T staging tiles (double-buffered by unit parity);
        # row 64 is the constant fft bias row, written once here
        ys = []
        for par in range(2):
            yra = const_pool.tile([65, 128, 4], bf16, tag=f"yra{par}")
            yia = const_pool.tile([65, 128, 4], bf16, tag=f"yia{par}")
            yrb = const_pool.tile([65, 128, 4], bf16, tag=f"yrb{par}")
            yib = const_pool.tile([65, 128, 4], bf16, tag=f"yib{par}")
            nc.vector.memset(yra[64:65], 1.0)
            nc.vector.memset(yrb[64:65], 1.0)
            nc.vector.memset(yia[64:65], 0.0)
            nc.vector.memset(yib[64:65], 0.0)
            ys.append((yra, yia, yrb, yib))

        st = {}

        def stage_a1(b, g):
            xin = xpool.tile([128, 2, 2, 4, W], u16, tag="xin")
            nc.sync.dma_start(
                xin[:, 0], axp[b, g, 0].rearrange("o (p a) w -> p o a w", p=128)
            )
            nc.sync.dma_start(
                xin[:, 1], axp[b, g, 1].rearrange("o (p a) w -> p o a w", p=128)
            )
            st[(b, g)] = {"xin": xin}

        def stage_a2(b, g):
            # whole diff chain on DVE: no mid-chain cross-engine hops.
            # (max - min) >= 0 so no abs; u16 ops run at 2x DVE rate.
            s = st[(b, g)]
            xin = s.pop("xin")
            nc.vector.tensor_tensor(
                xin[:, 0, 1], xin[:, 0, 0], xin[:, 0, 1], Alu.subtract
            )
            nc.vector.tensor_tensor(
                xin[:, 1, 1], xin[:, 1, 0], xin[:, 1, 1], Alu.subtract
            )
            # d0 + d1 <= 64574 fits u16; then widen to i32
            nc.vector.tensor_tensor(xin[:, 0, 0], xin[:, 0, 1], xin[:, 1, 1], Alu.add)
            u2 = vpool.tile([128, 2, W], i32, tag="u2")
            nc.vector.tensor_tensor(
                u2[:], xin[:, 0, 0, 0:2], xin[:, 0, 0, 2:4], Alu.add
            )
            v = vpool.tile([128, W], i32, tag="scr512")
            nc.vector.tensor_tensor(v[:], u2[:, 0], u2[:, 1], Alu.add)
            drow = vpool.tile([128, W], f32, tag="drow")
            nc.gpsimd.partition_all_reduce(
                drow[:], v[:], channels=128, reduce_op=bass_isa.ReduceOp.add
            )
            s["drow"] = drow

        def stage_s1(b, g):
            s = st[(b, g)]
            nc.scalar.dma_start(
                dscr[b, g].rearrange("(a b) -> a b", a=1), s["drow"][0:1, :]
            )
            dcol = spool.tile([128, 4], f32, tag="dcol")
            nc.scalar.dma_start(dcol[:], dscr[b, g].rearrange("(a b) -> a b", a=128))
            s["dcol"] = dcol

        def stage_b(b, g):
            s = st[(b, g)]
            drow, dcol = s.pop("drow"), s.pop("dcol")
            rank = spool.tile([128, 4], f32, tag="rank")
            cmp = vpool.tile([128, W], f32, tag="scr512")
            for wq in range(4):
                nc.vector.tensor_scalar(
                    cmp[:], drow[:], dcol[:, wq : wq + 1], 0.0, Alu.is_lt,
                    Alu.add, accum_out=rank[:, wq : wq + 1],
                )
            mask = spool.tile([128, 4], u8, tag="mask")
            nc.vector.tensor_scalar(mask[:], rank[:], float(T), None, Alu.is_lt)
            vals = spool.tile([128, 4], f32, tag="vals")
            nc.vector.memset(vals[:], -1.0)
            nc.vector.copy_predicated(vals[:], mask[:], iota_f[:])
            s["vals"] = vals

        def stage_s2(b, g):
            s = st[(b, g)]
            nc.scalar.dma_start(
                vscr[b, g].rearrange("(a b) -> a b", a=128), s.pop("vals")[:]
            )
            v16 = spool.tile([16, 32], f32, tag="v16")
            nc.scalar.dma_start(v16[:], vscr[b, g].rearrange("(a b) -> a b", a=16))
            s["v16"] = v16

        def stage_c(b, g):
            s = st[(b, g)]
            sel_f = spool.tile([16, 4], f32, tag="self")
            nfound = spool.tile([1, 1], u32, tag="nf")
            nc.gpsimd.sparse_gather(sel_f[:], s.pop("v16")[:], num_found=nfound[:])
            sel16 = spool.tile([16, 4], i16, tag="sel16")
            nc.gpsimd.tensor_copy(sel16[:], sel_f[:])
            idx_a = ipool.tile([128, 16], i16, tag="idxa")
            # cols 0:8 = gx lists (c0 sel | c1 sel), cols 8:13 = gc (sel + bias W)
            nc.vector.tensor_scalar(
                idx_a[0:16, 0:4], sel16[:], 2 * g * W, None, Alu.add
            )
            nc.vector.tensor_scalar(
                idx_a[0:16, 4:8], sel16[:], (2 * g + 1) * W, None, Alu.add
            )
            nc.vector.tensor_copy(idx_a[0:16, 8:12], sel16[:])
            nc.vector.memset(idx_a[0:16, 12:16], 0)
            nc.vector.memset(idx_a[0:1, 12:13], W)
            s["idx"] = idx_a

        def stage_creps(b, g):
            idx_a = st[(b, g)]["idx"]
            nc.scalar.dma_start(idx_a[16:32, :], idx_a[0:16, :])
            nc.scalar.dma_start(idx_a[32:64, :], idx_a[0:32, :])
            nc.scalar.dma_start(idx_a[64:128, :], idx_a[0:64, :])

        def stage_c2(b, g):
            s = st[(b, g)]
            idx_a = s.pop("idx")
            gxt = gxpool.tile([128, 8, 128], bf16, tag="gx")
            gc = gcpool.tile([128, 1, 4 * W], bf16, tag="gc")
            nc.gpsimd.dma_gather(
                gxt[:], x16[b],
                idx_a[:, 0:8], num_idxs=128, num_idxs_reg=128,
                elem_size=2 * H, transpose=True,
            )
            nc.gpsimd.dma_gather(
                gc[:], cb_d[:], idx_a[:, 8:13],
                num_idxs=65, num_idxs_reg=65, elem_size=4 * W,
            )
            s["gx"] = gxt
            s["gc"] = gc

        def stage_d1(b, g, par):
            s = st[(b, g)]
            gxt = s.pop("gx")
            yr_ps = psum_y.tile([128, H], f32, tag="yrp")
            yi_ps = psum_y.tile([128, H], f32, tag="yip")
            # consecutive matmuls share the stationary tensor (one LDWEIGHTS
            # per lre/lim instead of per matmul)
            for hq in range(4):
                first = hq == 0
                last = hq == 3
                lre = gxt[:, hq, :]
                lim = gxt[:, 4 + hq, :]
                nc.tensor.matmul(
                    yr_ps[:], lre, sb_cos[:, hq, :], start=first, stop=False
                )
                nc.tensor.matmul(
                    yi_ps[:], lre, sb_sin[:, hq, :], start=first, stop=False
                )
                nc.tensor.matmul(
                    yr_ps[:], lim, sb_nsin[:, hq, :], start=False, stop=last
                )
                nc.tensor.matmul(
                    yi_ps[:], lim, sb_cos[:, hq, :], start=False, stop=last
                )
            yra, yia, yrb, yib = ys[par]
            rr = lambda ap: ap.rearrange("t (p j) -> t p j", j=4)
            nc.scalar.mul(yra[0:64], rr(yr_ps[0:64, :]), 1.0)
            nc.vector.tensor_copy(yia[0:64], rr(yi_ps[0:64, :]))
            nc.scalar.mul(yrb[0:64], rr(yr_ps[64:128, :]), 1.0)
            nc.vector.tensor_copy(yib[0:64], rr(yi_ps[64:128, :]))
            s["y"] = (yra, yia, yrb, yib)

        def stage_d2(b, g):
            s = st[(b, g)]
            gc = s.pop("gc")
            yra, yia, yrb, yib = s.pop("y")
            osb = opool.tile([128, 2, 4, 2 * W], bf16, tag="osb")
            evict_eng = [nc.scalar, nc.vector, nc.scalar, nc.vector,
                         nc.scalar, nc.vector, nc.scalar, nc.vector]
            for ci, (yrt, yit) in enumerate(((yra, yia), (yrb, yib))):
                for j in range(4):
                    o_ps = psum_o.tile([128, 2 * W], f32, tag="ops")
                    # yr over both halves first, then yi: one LDWEIGHTS each
                    for nh in range(2):
                        nsl = slice(nh * W, (nh + 1) * W)
                        nc.tensor.matmul(
                            o_ps[:, nsl], yrt[:, :, j],
                            gc[0:65, 0, nh * W : (nh + 1) * W],
                            start=True, stop=False,
                        )
                    for nh in range(2):
                        nsl = slice(nh * W, (nh + 1) * W)
                        nc.tensor.matmul(
                            o_ps[:, nsl], yit[:, :, j],
                            gc[0:65, 0, 2 * W + nh * W : 2 * W + (nh + 1) * W],
                            start=False, stop=True,
                        )
                    eng = evict_eng[ci * 4 + j]
                    if eng is nc.scalar:
                        nc.scalar.mul(osb[:, ci, j, :], o_ps[:], 1.0)
                    else:
                        eng.tensor_copy(osb[:, ci, j, :], o_ps[:])
            s["osb"] = osb

        def stage_out(b, g):
            s = st.pop((b, g))
            osb = s["osb"]
            c0 = 2 * g
            nc.sync.dma_start(
                out_d[b, c0].rearrange("(p j) w t -> p j (w t)", j=4), osb[:, 0]
            )
            nc.sync.dma_start(
                out_d[b, c0 + 1].rearrange("(p j) w t -> p j (w t)", j=4), osb[:, 1]
            )

        # Two dense global phases. Phase 1: input streaming + selection for all
        # units (short homogeneous engine streams, no PE/evict interference).
        # Phase 2: gathers + FFT matmuls + evicts + stores (dense PE stream,
        # sync DGE free for output). The phases overlap naturally at the
        # boundary since phase-2's first deps complete during phase-1's tail.
        units = [(b, g) for b in range(BPC) for g in range(G)]
        n = len(units)
        for i in range(n + 7):
            if i < n:
                stage_a1(*units[i])
            if 7 <= i < n + 7:
                stage_c2(*units[i - 7])
            if 6 <= i < n + 6:
                stage_creps(*units[i - 6])
            if 5 <= i < n + 5:
                stage_c(*units[i - 5])
            if 4 <= i < n + 4:
                stage_s2(*units[i - 4])
            if 3 <= i < n + 3:
                stage_b(*units[i - 3])
            if 2 <= i < n + 2:
                stage_s1(*units[i - 2])
            if 1 <= i < n + 1:
                stage_a2(*units[i - 1])
        for k in range(n + 2):
            if 2 <= k < n + 2:
                stage_out(*units[k - 2])
            if k < n:
                stage_d1(*units[k], k % 2)
            if 1 <= k < n + 1:
                stage_d2(*units[k - 1])

    nc.compile()
    return nc


def _get_nc():
    if "nc" not in _cache:
        _cache["nc"] = _build_kernel()
    return _cache["nc"]


def _make_in_maps(xr, xi):
    import ml_dtypes

    bf = ml_dtypes.bfloat16
    cos_i, sin_i, nsin_i, cboth = _cache.setdefault("consts", _build_consts())
    # quantized |x| pre-paired as (max, min) u16 for the diff phase
    SCALE = 5957.0
    q0 = np.round(np.abs(xr).reshape(B, G, 2, H, W) * SCALE).astype(np.int32)
    q1 = np.round(np.abs(xi).reshape(B, G, 2, H, W) * SCALE).astype(np.int32)
    axp = np.empty((B, G, 2, 2, H, W), np.uint16)
    axp[:, :, 0, 0] = np.maximum(q0[:, :, 0], q1[:, :, 1])
    axp[:, :, 0, 1] = np.minimum(q0[:, :, 0], q1[:, :, 1])
    axp[:, :, 1, 0] = np.maximum(q0[:, :, 1], q1[:, :, 0])
    axp[:, :, 1, 1] = np.minimum(q0[:, :, 1], q1[:, :, 0])
    # transposed bf16 copy: row (c*W + w) = [re column (H) | im column (H)]
    x16 = np.stack(
        [xr.transpose(0, 1, 3, 2), xi.transpose(0, 1, 3, 2)], axis=3
    ).astype(bf)
    x16 = np.ascontiguousarray(x16).reshape(B, C * W, 2 * H)
    in_maps = []
    for i in range(NCORES):
        sl = slice(i * BPC, (i + 1) * BPC)
        in_maps.append(
            {
                "axp": axp[sl],
                "x16": x16[sl],
                "cos_i": cos_i,
                "sin_i": sin_i,
                "nsin_i": nsin_i,
                "cboth": cboth,
            }
        )
    return in_maps


def kernel(x_real: np.ndarray, x_imag: np.ndarray) -> np.ndarray:
    from concourse.bass_utils import run_bass_kernel_spmd

    xr = np.ascontiguousarray(np.asarray(x_real, dtype=np.float32))
    xi = np.ascontiguousarray(np.asarray(x_imag, dtype=np.float32))
    nc = _get_nc()
    in_maps = _make_in_maps(xr, xi)
    res = run_bass_kernel_spmd(nc, in_maps, core_ids=list(range(NCORES)))
    outs = [res.results[i]["out"] for i in range(NCORES)]
    return np.concatenate(outs, axis=0).astype(np.float32)


if __name__ == "__main__":
    rng = np.random.RandomState(0)
    out = kernel(
        rng.randn(B, C, H, W).astype(np.float32),
        rng.randn(B, C, H, W).astype(np.float32),
    )
    print(out.shape, out.dtype)


# revision 51
# speedup vs baseline: 1.0618x; 1.0618x over previous
"""Angular-select masked-FFT kernel for Trainium2 (8 NeuronCores, data-parallel over batch).

Math: per (b, g): diff[w] = sum_h ||re0|-|im1|| + ||re1|-|im0||; select 64 smallest w;
out = fft_w(ifft_h(x masked to sel columns)) + 0.5, emitted as interleaved re/im.

Two dense global phases over the 8 (b,g) units per core; stages are emitted
oldest-unit-first so the in-order engine streams never head-of-line block, the
SP DGE carries only input loads + output stores, and the ACT DGE carries the
small scratch round trips.

Phase 1 (input streaming + selection), unit-pipelined 7 deep:
  u+0  input load (sync DGE): u16-quantized (max,min) |x| pairs, 8KB/partition
       descriptors. Quantization (round(|x|*5957)) keeps every diff integer-
       exact (< 2^24, verified 0 selection flips, min boundary gap 21 ulp) at
       half the f32 traffic, and the host-side (max,min) ordering makes
       ||a|-|b|| a single u16 subtract (2x DVE rate, no abs pass).
  u+1  whole diff chain on DVE (no cross-engine hops): 2 subs + widening tree
       adds, then gpsimd partition_all_reduce -> broadcast diff row (f32-exact)
  u+2  dscr write + dcol read (ACT DGE round trip)
  u+3  rank[w] = #{d[w'] < d[w]} via is_lt+accum (DVE); mask = rank < 64
  u+4  vscr write + v16 read (ACT DGE)
  u+5  sparse_gather compacts selected w ids (gpsimd); idx lists (DVE)
  u+6  idx replication to 128 partitions (ACT DGE)

Phase 2 (gathers + FFTs as matmuls), unit-pipelined 3 deep:
  k+0  gathers (gpsimd DGE): gx 128 rows x 2KB transposed (re|im per row ->
       h-chunk lhsT layout), gc 65 rows x 4KB (C1|C2 DFT rows + bias row
       folding the +0.5)
  k+1  stage B ifft_h: both channels packed in one 128-row PSUM pair, 16
       matmuls, stationary operand reused across consecutive matmuls (half the
       LDWEIGHTS); contiguous evict to bf16 Y^T tiles [65,128,4] (h'=4p+j),
       bias rows persistent in parity-alternated tiles
  k+2  stage C fft_w: contraction 65 (64 sel + bias row), interleaved re/im
       output; evict to bf16 staging laid out so the store writes
       8KB/partition descriptors
  k+3  output store (sync DGE)

Output is bf16, upcast to f32 on host (error budget 2e-2 >> bf16 rounding).
"""

import os
import sys
from contextlib import ExitStack

import numpy as np

sys.path.insert(0, "/opt/trn_rl_repo")

B, C, H, W = 32, 4, 512, 512
T = 64
G = 2
NCORES = 8
BPC = B // NCORES  # samples per core

_cache = {}


def _build_consts():
    import ml_dtypes

    bf = ml_dtypes.bfloat16
    h = np.arange(H, dtype=np.float64)
    th = 2.0 * np.pi * np.outer(h, h) / H
    cos_i = (np.cos(th) / H).astype(np.float32)
    sin_i = (np.sin(th) / H).astype(np.float32)
    w = np.arange(W, dtype=np.float64)
    tw = 2.0 * np.pi * np.outer(w, w) / W
    fr = np.cos(tw).astype(np.float32)
    fi = (-np.sin(tw)).astype(np.float32)
    # row w = [C1_w (1024 interleaved re/im), C2_w (1024)]; row W = bias (+0.5 re)
    cb = np.zeros((W + 1, 2, 2 * W), np.float32)
    cb[:W, 0, 0::2] = fr
    cb[:W, 0, 1::2] = fi
    cb[:W, 1, 0::2] = -fi
    cb[:W, 1, 1::2] = fr
    cb[W, 0, 0::2] = 0.5
    return (
        cos_i.astype(bf),
        sin_i.astype(bf),
        (-sin_i).astype(bf),
        np.ascontiguousarray(cb.reshape(W + 1, 4 * W)).astype(bf),
    )


def _build_kernel():
    import concourse.bass as bass
    import concourse.tile as tile
    from concourse import bacc, bass_isa, mybir

    f32 = mybir.dt.float32
    bf16 = mybir.dt.bfloat16
    i16 = mybir.dt.int16
    i32 = mybir.dt.int32
    u32 = mybir.dt.uint32
    u16 = mybir.dt.uint16
    u8 = mybir.dt.uint8
    Alu = mybir.AluOpType
    Act = mybir.ActivationFunctionType
    MASK = 0x7FFFFFFF

    nc = bacc.Bacc("TRN2", target_bir_lowering=False, debug=False, num_devices=NCORES)

    # quantized |x| pre-paired as (max, min): pair0 = (|re0|,|im1|), pair1 =
    # (|re1|,|im0|); values = round(|x| * 5957) in u16 so the per-element
    # ||a|-|b|| is ONE subtract and all integer sums stay exact (< 2^24)
    axp = nc.dram_tensor("axp", [BPC, G, 2, 2, H, W], u16, kind="ExternalInput").ap()
    # transposed bf16: row (c*W + w) = [x_re[b,c,:,w] (H), x_im[b,c,:,w] (H)]
    x16 = nc.dram_tensor("x16", [BPC, C * W, 2 * H], bf16, kind="ExternalInput").ap()
    cos_d = nc.dram_tensor("cos_i", [H, H], bf16, kind="ExternalInput").ap()
    sin_d = nc.dram_tensor("sin_i", [H, H], bf16, kind="ExternalInput").ap()
    nsin_d = nc.dram_tensor("nsin_i", [H, H], bf16, kind="ExternalInput").ap()
    cb_d = nc.dram_tensor("cboth", [W + 1, 4 * W], bf16, kind="ExternalInput").ap()
    out_d = nc.dram_tensor("out", [BPC, C, H, W, 2], bf16, kind="ExternalOutput").ap()
    dscr = nc.dram_tensor("dscr", [BPC, G, 512], f32).ap()
    vscr = nc.dram_tensor("vscr", [BPC, G, 512], f32).ap()

    with tile.TileContext(nc) as tc, ExitStack() as ctx:
        const_pool = ctx.enter_context(tc.tile_pool(name="consts", bufs=1))
        xpool = ctx.enter_context(tc.tile_pool(name="x", bufs=3))
        vpool = ctx.enter_context(tc.tile_pool(name="vsmall", bufs=4))
        spool = ctx.enter_context(tc.tile_pool(name="small", bufs=4))
        ipool = ctx.enter_context(tc.tile_pool(name="idx", bufs=8))
        gxpool = ctx.enter_context(tc.tile_pool(name="gatherx", bufs=8))
        gcpool = ctx.enter_context(tc.tile_pool(name="gatherc", bufs=8))
        psum_y = ctx.enter_context(tc.tile_pool(name="psy", bufs=1, space="PSUM"))
        psum_o = ctx.enter_context(tc.tile_pool(name="pso", bufs=3, space="PSUM"))
        opool = ctx.enter_context(tc.tile_pool(name="ostage", bufs=3))

        # --- constants in SBUF ---
        sb_cos = const_pool.tile([128, 4, H], bf16)
        sb_sin = const_pool.tile([128, 4, H], bf16)
        sb_nsin = const_pool.tile([128, 4, H], bf16)
        nc.sync.dma_start(sb_cos[:], cos_d.rearrange("(a p) w -> p a w", p=128))
        nc.sync.dma_start(sb_sin[:], sin_d.rearrange("(a p) w -> p a w", p=128))
        nc.sync.dma_start(sb_nsin[:], nsin_d.rearrange("(a p) w -> p a w", p=128))
        iota_i = const_pool.tile([128, 4], i32)
        iota_f = const_pool.tile([128, 4], f32)
        nc.gpsimd.iota(iota_i[:], pattern=[[1, 4]], base=0, channel_multiplier=4)
        nc.vector.tensor_copy(iota_f[:], iota_i[:])
        # persistent Y^T staging tiles (double-buffered by unit parity);
        # row 64 is the constant fft bias row, written once here
        ys = []
        for par in range(2):
            yra = const_pool.tile([65, 128, 4], bf16, tag=f"yra{par}")
            yia = const_pool.tile([65, 128, 4], bf16, tag=f"yia{par}")
            yrb = const_pool.tile([65, 128, 4], bf16, tag=f"yrb{par}")
            yib = const_pool.tile([65, 128, 4], bf16, tag=f"yib{par}")
            nc.vector.memset(yra[64:65], 1.0)
            nc.vector.memset(yrb[64:65], 1.0)
            nc.vector.memset(yia[64:65], 0.0)
            nc.vector.memset(yib[64:65], 0.0)
            ys.append((yra, yia, yrb, yib))

        st = {}

        def stage_a1(b, g):
            xin = xpool.tile([128, 2, 2, 4, W], u16, tag="xin")
            nc.sync.dma_start(
                xin[:, 0], axp[b, g, 0].rearrange("o (p a) w -> p o a w", p=128)
            )
            nc.sync.dma_start(
                xin[:, 1], axp[b, g, 1].rearrange("o (p a) w -> p o a w", p=128)
            )
            st[(b, g)] = {"xin": xin}

        def stage_a2(b, g):
            # whole diff chain on DVE: no mid-chain cross-engine hops.
            # (max - min) >= 0 so no abs; u16 ops run at 2x DVE rate.
            s = st[(b, g)]
            xin = s.pop("xin")
            nc.vector.tensor_tensor(
                xin[:, 0, 1], xin[:, 0, 0], xin[:, 0, 1], Alu.subtract
            )
            nc.vector.tensor_tensor(
                xin[:, 1, 1], xin[:, 1, 0], xin[:, 1, 1], Alu.subtract
            )
            # d0 + d1 <= 64574 fits u16; then widen to i32
            nc.vector.tensor_tensor(xin[:, 0, 0], xin[:, 0, 1], xin[:, 1, 1], Alu.add)
            u2 = vpool.tile([128, 2, W], i32, tag="u2")
            nc.vector.tensor_tensor(
                u2[:], xin[:, 0, 0, 0:2], xin[:, 0, 0, 2:4], Alu.add
            )
            v = vpool.tile([128, W], i32, tag="scr512")
            nc.vector.tensor_tensor(v[:], u2[:, 0], u2[:, 1], Alu.add)
            drow = vpool.tile([128, W], f32, tag="drow")
            nc.gpsimd.partition_all_reduce(
                drow[:], v[:], channels=128, reduce_op=bass_isa.ReduceOp.add
            )
            s["drow"] = drow

        def stage_s1(b, g):
            s = st[(b, g)]
            nc.scalar.dma_start(
                dscr[b, g].rearrange("(a b) -> a b", a=1), s["drow"][0:1, :]
            )
            dcol = spool.tile([128, 4], f32, tag="dcol")
            nc.scalar.dma_start(dcol[:], dscr[b, g].rearrange("(a b) -> a b", a=128))
            s["dcol"] = dcol

        def stage_b(b, g):
            s = st[(b, g)]
            drow, dcol = s.pop("drow"), s.pop("dcol")
            rank = spool.tile([128, 4], f32, tag="rank")
            cmp = vpool.tile([128, W], f32, tag="scr512")
            for wq in range(4):
                nc.vector.tensor_scalar(
                    cmp[:], drow[:], dcol[:, wq : wq + 1], 0.0, Alu.is_lt,
                    Alu.add, accum_out=rank[:, wq : wq + 1],
                )
            mask = spool.tile([128, 4], u8, tag="mask")
            nc.vector.tensor_scalar(mask[:], rank[:], float(T), None, Alu.is_lt)
            vals = spool.tile([128, 4], f32, tag="vals")
            nc.vector.memset(vals[:], -1.0)
            nc.vector.copy_predicated(vals[:], mask[:], iota_f[:])
            s["vals"] = vals

        def stage_s2(b, g):
            s = st[(b, g)]
            nc.scalar.dma_start(
                vscr[b, g].rearrange("(a b) -> a b", a=128), s.pop("vals")[:]
            )
            v16 = spool.tile([16, 32], f32, tag="v16")
            nc.scalar.dma_start(v16[:], vscr[b, g].rearrange("(a b) -> a b", a=16))
            s["v16"] = v16

        def stage_c(b, g):
            s = st[(b, g)]
            sel_f = spool.tile([16, 4], f32, tag="self")
            nfound = spool.tile([1, 1], u32, tag="nf")
            nc.gpsimd.sparse_gather(sel_f[:], s.pop("v16")[:], num_found=nfound[:])
            sel16 = spool.tile([16, 4], i16, tag="sel16")
            nc.gpsimd.tensor_copy(sel16[:], sel_f[:])
            idx_a = ipool.tile([128, 16], i16, tag="idxa")
            # cols 0:8 = gx lists (c0 sel | c1 sel), cols 8:13 = gc (sel + bias W)
            nc.vector.tensor_scalar(
                idx_a[0:16, 0:4], sel16[:], 2 * g * W, None, Alu.add
            )
            nc.vector.tensor_scalar(
                idx_a[0:16, 4:8], sel16[:], (2 * g + 1) * W, None, Alu.add
            )
            nc.vector.tensor_copy(idx_a[0:16, 8:12], sel16[:])
            nc.vector.memset(idx_a[0:16, 12:16], 0)
            nc.vector.memset(idx_a[0:1, 12:13], W)
            nc.scalar.dma_start(idx_a[16:32, :], idx_a[0:16, :])
            nc.scalar.dma_start(idx_a[32:64, :], idx_a[0:32, :])
            nc.scalar.dma_start(idx_a[64:128, :], idx_a[0:64, :])
            s["idx"] = idx_a

        def stage_c2(b, g):
            s = st[(b, g)]
            idx_a = s.pop("idx")
            gxt = gxpool.tile([128, 8, 128], bf16, tag="gx")
            gc = gcpool.tile([128, 1, 4 * W], bf16, tag="gc")
            nc.gpsimd.dma_gather(
                gxt[:], x16[b],
                idx_a[:, 0:8], num_idxs=128, num_idxs_reg=128,
                elem_size=2 * H, transpose=True,
            )
            nc.gpsimd.dma_gather(
                gc[:], cb_d[:], idx_a[:, 8:13],
                num_idxs=65, num_idxs_reg=65, elem_size=4 * W,
            )
            s["gx"] = gxt
            s["gc"] = gc

        def stage_d1(b, g, par):
            s = st[(b, g)]
            gxt = s.pop("gx")
            yr_ps = psum_y.tile([128, H], f32, tag="yrp")
            yi_ps = psum_y.tile([128, H], f32, tag="yip")
            # consecutive matmuls share the stationary tensor (one LDWEIGHTS
            # per lre/lim instead of per matmul)
            for hq in range(4):
                first = hq == 0
                last = hq == 3
                lre = gxt[:, hq, :]
                lim = gxt[:, 4 + hq, :]
                nc.tensor.matmul(
                    yr_ps[:], lre, sb_cos[:, hq, :], start=first, stop=False
                )
                nc.tensor.matmul(
                    yi_ps[:], lre, sb_sin[:, hq, :], start=first, stop=False
                )
                nc.tensor.matmul(
                    yr_ps[:], lim, sb_nsin[:, hq, :], start=False, stop=last
                )
                nc.tensor.matmul(
                    yi_ps[:], lim, sb_cos[:, hq, :], start=False, stop=last
                )
            yra, yia, yrb, yib = ys[par]
            rr = lambda ap: ap.rearrange("t (p j) -> t p j", j=4)
            nc.scalar.mul(yra[0:64], rr(yr_ps[0:64, :]), 1.0)
            nc.scalar.mul(yia[0:64], rr(yi_ps[0:64, :]), 1.0)
            nc.scalar.mul(yrb[0:64], rr(yr_ps[64:128, :]), 1.0)
            nc.scalar.mul(yib[0:64], rr(yi_ps[64:128, :]), 1.0)
            s["y"] = (yra, yia, yrb, yib)

        def stage_d2(b, g):
            s = st[(b, g)]
            gc = s.pop("gc")
            yra, yia, yrb, yib = s.pop("y")
            osb = opool.tile([128, 2, 4, 2 * W], bf16, tag="osb")
            evict_eng = [nc.scalar, nc.vector, nc.scalar, nc.vector,
                         nc.scalar, nc.vector, nc.scalar, nc.vector]
            for ci, (yrt, yit) in enumerate(((yra, yia), (yrb, yib))):
                for j in range(4):
                    o_ps = psum_o.tile([128, 2 * W], f32, tag="ops")
                    # yr over both halves first, then yi: one LDWEIGHTS each
                    for nh in range(2):
                        nsl = slice(nh * W, (nh + 1) * W)
                        nc.tensor.matmul(
                            o_ps[:, nsl], yrt[:, :, j],
                            gc[0:65, 0, nh * W : (nh + 1) * W],
                            start=True, stop=False,
                        )
                    for nh in range(2):
                        nsl = slice(nh * W, (nh + 1) * W)
                        nc.tensor.matmul(
                            o_ps[:, nsl], yit[:, :, j],
                            gc[0:65, 0, 2 * W + nh * W : 2 * W + (nh + 1) * W],
                            start=False, stop=True,
                        )
                    eng = evict_eng[ci * 4 + j]
                    if eng is nc.scalar:
                        nc.scalar.mul(osb[:, ci, j, :], o_ps[:], 1.0)
                    else:
                        eng.tensor_copy(osb[:, ci, j, :], o_ps[:])
            s["osb"] = osb

        def stage_out(b, g):
            s = st.pop((b, g))
            osb = s["osb"]
            c0 = 2 * g
            nc.sync.dma_start(
                out_d[b, c0].rearrange("(p j) w t -> p j (w t)", j=4), osb[:, 0]
            )
            nc.sync.dma_start(
                out_d[b, c0 + 1].rearrange("(p j) w t -> p j (w t)", j=4), osb[:, 1]
            )

        # Two dense global phases. Phase 1: input streaming + selection for all
        # units (short homogeneous engine streams, no PE/evict interference).
        # Phase 2: gathers + FFT matmuls + evicts + stores (dense PE stream,
        # sync DGE free for output). The phases overlap naturally at the
        # boundary since phase-2's first deps complete during phase-1's tail.
        units = [(b, g) for b in range(BPC) for g in range(G)]
        n = len(units)
        for i in range(n + 6):
            if i < n:
                stage_a1(*units[i])
            if 6 <= i < n + 6:
                stage_c2(*units[i - 6])
            if 5 <= i < n + 5:
                stage_c(*units[i - 5])
            if 4 <= i < n + 4:
                stage_s2(*units[i - 4])
            if 3 <= i < n + 3:
                stage_b(*units[i - 3])
            if 2 <= i < n + 2:
                stage_s1(*units[i - 2])
            if 1 <= i < n + 1:
                stage_a2(*units[i - 1])
        for k in range(n + 2):
            if 2 <= k < n + 2:
                stage_out(*units[k - 2])
            if k < n:
                stage_d1(*units[k], k % 2)
            if 1 <= k < n + 1:
                stage_d2(*units[k - 1])

    nc.compile()
    return nc


def _get_nc():
    if "nc" not in _cache:
        _cache["nc"] = _build_kernel()
    return _cache["nc"]


def _make_in_maps(xr, xi):
    import ml_dtypes

    bf = ml_dtypes.bfloat16
    cos_i, sin_i, nsin_i, cboth = _cache.setdefault("consts", _build_consts())
    # quantized |x| pre-paired as (max, min) u16 for the diff phase
    SCALE = 5957.0
    q0 = np.round(np.abs(xr).reshape(B, G, 2, H, W) * SCALE).astype(np.int32)
    q1 = np.round(np.abs(xi).reshape(B, G, 2, H, W) * SCALE).astype(np.int32)
    axp = np.empty((B, G, 2, 2, H, W), np.uint16)
    axp[:, :, 0, 0] = np.maximum(q0[:, :, 0], q1[:, :, 1])
    axp[:, :, 0, 1] = np.minimum(q0[:, :, 0], q1[:, :, 1])
    axp[:, :, 1, 0] = np.maximum(q0[:, :, 1], q1[:, :, 0])
    axp[:, :, 1, 1] = np.minimum(q0[:, :, 1], q1[:, :, 0])
    # transposed bf16 copy: row (c*W + w) = [re column (H) | im column (H)]
    x16 = np.stack(
        [xr.transpose(0, 1, 3, 2), xi.transpose(0, 1, 3, 2)], axis=3
    ).astype(bf)
    x16 = np.ascontiguousarray(x16).reshape(B, C * W, 2 * H)
    in_maps = []
    for i in range(NCORES):
        sl = slice(i * BPC, (i + 1) * BPC)
        in_maps.append(
            {
                "axp": axp[sl],
                "x16": x16[sl],
                "cos_i": cos_i,
                "sin_i": sin_i,
                "nsin_i": nsin_i,
                "cboth": cboth,
            }
        )
    return in_maps


def kernel(x_real: np.ndarray, x_imag: np.ndarray) -> np.ndarray:
    from concourse.bass_utils import run_bass_kernel_spmd

    xr = np.ascontiguousarray(np.asarray(x_real, dtype=np.float32))
    xi = np.ascontiguousarray(np.asarray(x_imag, dtype=np.float32))
    nc = _get_nc()
    in_maps = _make_in_maps(xr, xi)
    res = run_bass_kernel_spmd(nc, in_maps, core_ids=list(range(NCORES)))
    outs = [res.results[i]["out"] for i in range(NCORES)]
    return np.concatenate(outs, axis=0).astype(np.float32)


if __name__ == "__main__":
    rng = np.random.RandomState(0)
    out = kernel(
        rng.randn(B, C, H, W).astype(np.float32),
        rng.randn(B, C, H, W).astype(np.float32),
    )
    print(out.shape, out.dtype)


# revision 52
# speedup vs baseline: 1.1552x; 1.0880x over previous
"""Angular-select masked-FFT kernel for Trainium2 (8 NeuronCores, data-parallel over batch).

Math: per (b, g): diff[w] = sum_h ||re0|-|im1|| + ||re1|-|im0||; select 64 smallest w;
out = fft_w(ifft_h(x masked to sel columns)) + 0.5, emitted as interleaved re/im.

Two dense global phases over the 8 (b,g) units per core; stages are emitted
oldest-unit-first so the in-order engine streams never head-of-line block, the
SP DGE carries only input loads + output stores, and the ACT DGE carries the
small scratch round trips.

Phase 1 (input streaming + selection), unit-pipelined 7 deep:
  u+0  input load (sync DGE): u16-quantized (max,min) |x| pairs, 8KB/partition
       descriptors. Quantization (round(|x|*5957)) keeps every diff integer-
       exact (< 2^24, verified 0 selection flips, min boundary gap 21 ulp) at
       half the f32 traffic, and the host-side (max,min) ordering makes
       ||a|-|b|| a single u16 subtract (2x DVE rate, no abs pass).
  u+1  whole diff chain on DVE (no cross-engine hops): 2 subs + widening tree
       adds, then gpsimd partition_all_reduce -> broadcast diff row (f32-exact)
  u+2  dscr write + dcol read (ACT DGE round trip)
  u+3  rank[w] = #{d[w'] < d[w]} via is_lt+accum (DVE); mask = rank < 64
  u+4  vscr write + v16 read (ACT DGE)
  u+5  sparse_gather compacts selected w ids (gpsimd); idx lists (DVE)
  u+6  idx replication to 128 partitions (ACT DGE)

Phase 2 (gathers + FFTs as matmuls), unit-pipelined 3 deep:
  k+0  gathers (gpsimd DGE): gx 128 rows x 2KB transposed (re|im per row ->
       h-chunk lhsT layout), gc 65 rows x 4KB (C1|C2 DFT rows + bias row
       folding the +0.5)
  k+1  stage B ifft_h: both channels packed in one 128-row PSUM pair, 16
       matmuls, stationary operand reused across consecutive matmuls (half the
       LDWEIGHTS); contiguous evict to bf16 Y^T tiles [65,128,4] (h'=4p+j),
       bias rows persistent in parity-alternated tiles
  k+2  stage C fft_w: contraction 65 (64 sel + bias row), interleaved re/im
       output; evict to bf16 staging laid out so the store writes
       8KB/partition descriptors
  k+3  output store (sync DGE)

Output is bf16, upcast to f32 on host (error budget 2e-2 >> bf16 rounding).
"""

import os
import sys
from contextlib import ExitStack

import numpy as np

sys.path.insert(0, "/opt/trn_rl_repo")

B, C, H, W = 32, 4, 512, 512
T = 64
G = 2
NCORES = 8
BPC = B // NCORES  # samples per core

_cache = {}


def _build_consts():
    import ml_dtypes

    bf = ml_dtypes.bfloat16
    h = np.arange(H, dtype=np.float64)
    th = 2.0 * np.pi * np.outer(h, h) / H
    cos_i = (np.cos(th) / H).astype(np.float32)
    sin_i = (np.sin(th) / H).astype(np.float32)
    w = np.arange(W, dtype=np.float64)
    tw = 2.0 * np.pi * np.outer(w, w) / W
    fr = np.cos(tw).astype(np.float32)
    fi = (-np.sin(tw)).astype(np.float32)
    # row w = [C1_w (1024 interleaved re/im), C2_w (1024)]; row W = bias (+0.5 re)
    cb = np.zeros((W + 1, 2, 2 * W), np.float32)
    cb[:W, 0, 0::2] = fr
    cb[:W, 0, 1::2] = fi
    cb[:W, 1, 0::2] = -fi
    cb[:W, 1, 1::2] = fr
    cb[W, 0, 0::2] = 0.5
    return (
        cos_i.astype(bf),
        sin_i.astype(bf),
        (-sin_i).astype(bf),
        np.ascontiguousarray(cb.reshape(W + 1, 4 * W)).astype(bf),
    )


def _build_kernel():
    import concourse.bass as bass
    import concourse.tile as tile
    from concourse import bacc, bass_isa, mybir

    f32 = mybir.dt.float32
    bf16 = mybir.dt.bfloat16
    i16 = mybir.dt.int16
    i32 = mybir.dt.int32
    u32 = mybir.dt.uint32
    u16 = mybir.dt.uint16
    u8 = mybir.dt.uint8
    Alu = mybir.AluOpType
    Act = mybir.ActivationFunctionType
    MASK = 0x7FFFFFFF

    nc = bacc.Bacc("TRN2", target_bir_lowering=False, debug=False, num_devices=NCORES)

    # quantized |x| pre-paired as (max, min): pair0 = (|re0|,|im1|), pair1 =
    # (|re1|,|im0|); values = round(|x| * 5957) in u16 so the per-element
    # ||a|-|b|| is ONE subtract and all integer sums stay exact (< 2^24)
    axp = nc.dram_tensor("axp", [BPC, G, 2, 2, H, W], u16, kind="ExternalInput").ap()
    # transposed bf16: row (c*W + w) = [x_re[b,c,:,w] (H), x_im[b,c,:,w] (H)]
    x16 = nc.dram_tensor("x16", [BPC, C * W, 2 * H], bf16, kind="ExternalInput").ap()
    cos_d = nc.dram_tensor("cos_i", [H, H], bf16, kind="ExternalInput").ap()
    sin_d = nc.dram_tensor("sin_i", [H, H], bf16, kind="ExternalInput").ap()
    nsin_d = nc.dram_tensor("nsin_i", [H, H], bf16, kind="ExternalInput").ap()
    cb_d = nc.dram_tensor("cboth", [W + 1, 4 * W], bf16, kind="ExternalInput").ap()
    out_d = nc.dram_tensor("out", [BPC, C, H, W, 2], bf16, kind="ExternalOutput").ap()
    dscr = nc.dram_tensor("dscr", [BPC, G, 512], f32).ap()
    vscr = nc.dram_tensor("vscr", [BPC, G, 512], f32).ap()

    with tile.TileContext(nc) as tc, ExitStack() as ctx:
        const_pool = ctx.enter_context(tc.tile_pool(name="consts", bufs=1))
        xpool = ctx.enter_context(tc.tile_pool(name="x", bufs=3))
        vpool = ctx.enter_context(tc.tile_pool(name="vsmall", bufs=4))
        spool = ctx.enter_context(tc.tile_pool(name="small", bufs=4))
        ipool = ctx.enter_context(tc.tile_pool(name="idx", bufs=8))
        gxpool = ctx.enter_context(tc.tile_pool(name="gatherx", bufs=8))
        gcpool = ctx.enter_context(tc.tile_pool(name="gatherc", bufs=8))
        psum_y = ctx.enter_context(tc.tile_pool(name="psy", bufs=1, space="PSUM"))
        psum_o = ctx.enter_context(tc.tile_pool(name="pso", bufs=3, space="PSUM"))
        opool = ctx.enter_context(tc.tile_pool(name="ostage", bufs=3))

        # --- constants in SBUF ---
        sb_cos = const_pool.tile([128, 4, H], bf16)
        sb_sin = const_pool.tile([128, 4, H], bf16)
        sb_nsin = const_pool.tile([128, 4, H], bf16)
        nc.sync.dma_start(sb_cos[:], cos_d.rearrange("(a p) w -> p a w", p=128))
        nc.sync.dma_start(sb_sin[:], sin_d.rearrange("(a p) w -> p a w", p=128))
        nc.sync.dma_start(sb_nsin[:], nsin_d.rearrange("(a p) w -> p a w", p=128))
        iota_i = const_pool.tile([128, 4], i32)
        iota_f = const_pool.tile([128, 4], f32)
        nc.gpsimd.iota(iota_i[:], pattern=[[1, 4]], base=0, channel_multiplier=4)
        nc.vector.tensor_copy(iota_f[:], iota_i[:])
        # persistent Y^T staging tiles (double-buffered by unit parity);
        # row 64 is the constant fft bias row, written once here
        ys = []
        for par in range(2):
            yra = const_pool.tile([65, 128, 4], bf16, tag=f"yra{par}")
            yia = const_pool.tile([65, 128, 4], bf16, tag=f"yia{par}")
            yrb = const_pool.tile([65, 128, 4], bf16, tag=f"yrb{par}")
            yib = const_pool.tile([65, 128, 4], bf16, tag=f"yib{par}")
            nc.vector.memset(yra[64:65], 1.0)
            nc.vector.memset(yrb[64:65], 1.0)
            nc.vector.memset(yia[64:65], 0.0)
            nc.vector.memset(yib[64:65], 0.0)
            ys.append((yra, yia, yrb, yib))

        st = {}

        def stage_a1(b, g):
            xin = xpool.tile([128, 2, 2, 4, W], u16, tag="xin")
            nc.sync.dma_start(
                xin[:, 0], axp[b, g, 0].rearrange("o (p a) w -> p o a w", p=128)
            )
            nc.sync.dma_start(
                xin[:, 1], axp[b, g, 1].rearrange("o (p a) w -> p o a w", p=128)
            )
            st[(b, g)] = {"xin": xin}

        def stage_a2(b, g):
            # whole diff chain on DVE: no mid-chain cross-engine hops.
            # (max - min) >= 0 so no abs; u16 ops run at 2x DVE rate.
            s = st[(b, g)]
            xin = s.pop("xin")
            nc.vector.tensor_tensor(
                xin[:, 0, 1], xin[:, 0, 0], xin[:, 0, 1], Alu.subtract
            )
            nc.vector.tensor_tensor(
                xin[:, 1, 1], xin[:, 1, 0], xin[:, 1, 1], Alu.subtract
            )
            # d0 + d1 <= 64574 fits u16; then widen to i32
            nc.vector.tensor_tensor(xin[:, 0, 0], xin[:, 0, 1], xin[:, 1, 1], Alu.add)
            u2 = vpool.tile([128, 2, W], i32, tag="u2")
            nc.vector.tensor_tensor(
                u2[:], xin[:, 0, 0, 0:2], xin[:, 0, 0, 2:4], Alu.add
            )
            v = vpool.tile([128, W], i32, tag="scr512")
            nc.vector.tensor_tensor(v[:], u2[:, 0], u2[:, 1], Alu.add)
            drow = vpool.tile([128, W], f32, tag="drow")
            nc.gpsimd.partition_all_reduce(
                drow[:], v[:], channels=128, reduce_op=bass_isa.ReduceOp.add
            )
            s["drow"] = drow

        def stage_s1(b, g):
            s = st[(b, g)]
            nc.scalar.dma_start(
                dscr[b, g].rearrange("(a b) -> a b", a=1), s["drow"][0:1, :]
            )
            dcol = spool.tile([128, 4], f32, tag="dcol")
            nc.scalar.dma_start(dcol[:], dscr[b, g].rearrange("(a b) -> a b", a=128))
            s["dcol"] = dcol

        def stage_b(b, g):
            s = st[(b, g)]
            drow, dcol = s.pop("drow"), s.pop("dcol")
            rank = spool.tile([128, 4], f32, tag="rank")
            cmp = vpool.tile([128, W], f32, tag="scr512")
            for wq in range(4):
                nc.vector.tensor_scalar(
                    cmp[:], drow[:], dcol[:, wq : wq + 1], 0.0, Alu.is_lt,
                    Alu.add, accum_out=rank[:, wq : wq + 1],
                )
            mask = spool.tile([128, 4], u8, tag="mask")
            nc.vector.tensor_scalar(mask[:], rank[:], float(T), None, Alu.is_lt)
            vals = spool.tile([128, 4], f32, tag="vals")
            nc.vector.memset(vals[:], -1.0)
            nc.vector.copy_predicated(vals[:], mask[:], iota_f[:])
            s["vals"] = vals

        def stage_s2(b, g):
            s = st[(b, g)]
            nc.scalar.dma_start(
                vscr[b, g].rearrange("(a b) -> a b", a=128), s.pop("vals")[:]
            )
            v16 = spool.tile([16, 32], f32, tag="v16")
            nc.scalar.dma_start(v16[:], vscr[b, g].rearrange("(a b) -> a b", a=16))
            s["v16"] = v16

        def stage_c(b, g):
            s = st[(b, g)]
            sel_f = spool.tile([16, 4], f32, tag="self")
            nfound = spool.tile([1, 1], u32, tag="nf")
            nc.gpsimd.sparse_gather(sel_f[:], s.pop("v16")[:], num_found=nfound[:])
            sel16 = spool.tile([16, 4], i16, tag="sel16")
            nc.gpsimd.tensor_copy(sel16[:], sel_f[:])
            idx_a = ipool.tile([128, 16], i16, tag="idxa")
            # cols 0:8 = gx lists (c0 sel | c1 sel), cols 8:13 = gc (sel + bias W)
            nc.vector.tensor_scalar(
                idx_a[0:16, 0:4], sel16[:], 2 * g * W, None, Alu.add
            )
            nc.vector.tensor_scalar(
                idx_a[0:16, 4:8], sel16[:], (2 * g + 1) * W, None, Alu.add
            )
            nc.vector.tensor_copy(idx_a[0:16, 8:12], sel16[:])
            nc.vector.memset(idx_a[0:16, 12:16], 0)
            nc.vector.memset(idx_a[0:1, 12:13], W)
            nc.scalar.dma_start(idx_a[16:32, :], idx_a[0:16, :])
            nc.scalar.dma_start(idx_a[32:64, :], idx_a[0:32, :])
            nc.scalar.dma_start(idx_a[64:128, :], idx_a[0:64, :])
            s["idx"] = idx_a

        def stage_c2(b, g):
            s = st[(b, g)]
            idx_a = s.pop("idx")
            gxt = gxpool.tile([128, 8, 128], bf16, tag="gx")
            gc = gcpool.tile([128, 1, 4 * W], bf16, tag="gc")
            nc.gpsimd.dma_gather(
                gxt[:], x16[b],
                idx_a[:, 0:8], num_idxs=128, num_idxs_reg=128,
                elem_size=2 * H, transpose=True,
            )
            nc.gpsimd.dma_gather(
                gc[:], cb_d[:], idx_a[:, 8:13],
                num_idxs=65, num_idxs_reg=65, elem_size=4 * W,
            )
            s["gx"] = gxt
            s["gc"] = gc

        def stage_d1(b, g, par):
            s = st[(b, g)]
            gxt = s.pop("gx")
            yr_ps = psum_y.tile([128, H], f32, tag="yrp")
            yi_ps = psum_y.tile([128, H], f32, tag="yip")
            # consecutive matmuls share the stationary tensor (one LDWEIGHTS
            # per lre/lim instead of per matmul)
            for hq in range(4):
                first = hq == 0
                last = hq == 3
                lre = gxt[:, hq, :]
                lim = gxt[:, 4 + hq, :]
                nc.tensor.matmul(
                    yr_ps[:], lre, sb_cos[:, hq, :], start=first, stop=False
                )
                nc.tensor.matmul(
                    yi_ps[:], lre, sb_sin[:, hq, :], start=first, stop=False
                )
                nc.tensor.matmul(
                    yr_ps[:], lim, sb_nsin[:, hq, :], start=False, stop=last
                )
                nc.tensor.matmul(
                    yi_ps[:], lim, sb_cos[:, hq, :], start=False, stop=last
                )
            yra, yia, yrb, yib = ys[par]
            rr = lambda ap: ap.rearrange("t (p j) -> t p j", j=4)
            nc.scalar.mul(yra[0:64], rr(yr_ps[0:64, :]), 1.0)
            nc.scalar.mul(yia[0:64], rr(yi_ps[0:64, :]), 1.0)
            nc.scalar.mul(yrb[0:64], rr(yr_ps[64:128, :]), 1.0)
            nc.scalar.mul(yib[0:64], rr(yi_ps[64:128, :]), 1.0)
            s["y"] = (yra, yia, yrb, yib)

        def stage_d2(b, g):
            s = st[(b, g)]
            gc = s.pop("gc")
            yra, yia, yrb, yib = s.pop("y")
            osb = opool.tile([128, 2, 4, 2 * W], bf16, tag="osb")
            evict_eng = [nc.scalar, nc.vector, nc.scalar, nc.vector,
                         nc.scalar, nc.vector, nc.scalar, nc.vector]
            for ci, (yrt, yit) in enumerate(((yra, yia), (yrb, yib))):
                for j in range(4):
                    o_ps = psum_o.tile([128, 2 * W], f32, tag="ops")
                    # yr over both halves first, then yi: one LDWEIGHTS each
                    for nh in range(2):
                        nsl = slice(nh * W, (nh + 1) * W)
                        nc.tensor.matmul(
                            o_ps[:, nsl], yrt[:, :, j],
                            gc[0:65, 0, nh * W : (nh + 1) * W],
                            start=True, stop=False,
                        )
                    for nh in range(2):
                        nsl = slice(nh * W, (nh + 1) * W)
                        nc.tensor.matmul(
                            o_ps[:, nsl], yit[:, :, j],
                            gc[0:65, 0, 2 * W + nh * W : 2 * W + (nh + 1) * W],
                            start=False, stop=True,
                        )
                    eng = evict_eng[ci * 4 + j]
                    if eng is nc.scalar:
                        nc.scalar.mul(osb[:, ci, j, :], o_ps[:], 1.0)
                    else:
                        eng.tensor_copy(osb[:, ci, j, :], o_ps[:])
            s["osb"] = osb

        def stage_out(b, g):
            s = st.pop((b, g))
            osb = s["osb"]
            c0 = 2 * g
            nc.sync.dma_start(
                out_d[b, c0].rearrange("(p j) w t -> p j (w t)", j=4), osb[:, 0]
            )
            nc.sync.dma_start(
                out_d[b, c0 + 1].rearrange("(p j) w t -> p j (w t)", j=4), osb[:, 1]
            )

        # Two dense global phases. Phase 1: input streaming + selection for all
        # units (short homogeneous engine streams, no PE/evict interference).
        # Phase 2: gathers + FFT matmuls + evicts + stores (dense PE stream,
        # sync DGE free for output). The phases overlap naturally at the
        # boundary since phase-2's first deps complete during phase-1's tail.
        units = [(b, g) for b in range(BPC) for g in range(G)]
        n = len(units)
        for i in range(n + 5):
            if i < n:
                stage_a1(*units[i])
            if 5 <= i < n + 5:
                stage_c2(*units[i - 5])
            if 4 <= i < n + 4:
                stage_c(*units[i - 4])
            if 3 <= i < n + 3:
                stage_b(*units[i - 3])
                stage_s2(*units[i - 3])
            if 2 <= i < n + 2:
                stage_s1(*units[i - 2])
            if 1 <= i < n + 1:
                stage_a2(*units[i - 1])
        for k in range(n + 2):
            if 2 <= k < n + 2:
                stage_out(*units[k - 2])
            if k < n:
                stage_d1(*units[k], k % 2)
            if 1 <= k < n + 1:
                stage_d2(*units[k - 1])

    nc.compile()
    return nc


def _get_nc():
    if "nc" not in _cache:
        _cache["nc"] = _build_kernel()
    return _cache["nc"]


def _make_in_maps(xr, xi):
    import ml_dtypes

    bf = ml_dtypes.bfloat16
    cos_i, sin_i, nsin_i, cboth = _cache.setdefault("consts", _build_consts())
    # quantized |x| pre-paired as (max, min) u16 for the diff phase
    SCALE = 5957.0
    q0 = np.round(np.abs(xr).reshape(B, G, 2, H, W) * SCALE).astype(np.int32)
    q1 = np.round(np.abs(xi).reshape(B, G, 2, H, W) * SCALE).astype(np.int32)
    axp = np.empty((B, G, 2, 2, H, W), np.uint16)
    axp[:, :, 0, 0] = np.maximum(q0[:, :, 0], q1[:, :, 1])
    axp[:, :, 0, 1] = np.minimum(q0[:, :, 0], q1[:, :, 1])
    axp[:, :, 1, 0] = np.maximum(q0[:, :, 1], q1[:, :, 0])
    axp[:, :, 1, 1] = np.minimum(q0[:, :, 1], q1[:, :, 0])
    # transposed bf16 copy: row (c*W + w) = [re column (H) | im column (H)]
    x16 = np.stack(
        [xr.transpose(0, 1, 3, 2), xi.transpose(0, 1, 3, 2)], axis=3
    ).astype(bf)
    x16 = np.ascontiguousarray(x16).reshape(B, C * W, 2 * H)
    in_maps = []
    for i in range(NCORES):
        sl = slice(i * BPC, (i + 1) * BPC)
        in_maps.append(
            {
                "axp": axp[sl],
                "x16": x16[sl],
                "cos_i": cos_i,
                "sin_i": sin_i,
                "nsin_i": nsin_i,
                "cboth": cboth,
            }
        )
    return in_maps


def kernel(x_real: np.ndarray, x_imag: np.ndarray) -> np.ndarray:
    from concourse.bass_utils import run_bass_kernel_spmd

    xr = np.ascontiguousarray(np.asarray(x_real, dtype=np.float32))
    xi = np.ascontiguousarray(np.asarray(x_imag, dtype=np.float32))
    nc = _get_nc()
    in_maps = _make_in_maps(xr, xi)
    res = run_bass_kernel_spmd(nc, in_maps, core_ids=list(range(NCORES)))
    outs = [res.results[i]["out"] for i in range(NCORES)]
    return np.concatenate(outs, axis=0).astype(np.float32)


if __name__ == "__main__":
    rng = np.random.RandomState(0)
    out = kernel(
        rng.randn(B, C, H, W).astype(np.float32),
        rng.randn(B, C, H, W).astype(np.float32),
    )
    print(out.shape, out.dtype)
